# revision 5
# baseline (speedup 1.0000x reference)
"""Trainium2 Bass kernel for the DSS (Diagonal State Space) layer.

y = irfft(rfft(u, 2L) * rfft(K, 2L))[:L] + D*u, with K the length-L DSS kernel
derived from (Lambda, W, log_step) via a complex softmax.

Chunked diagonal-SSM scan, C=512 timesteps per chunk (4 tiles of 128):
  - intra-chunk: Toeplitz blocks T0..T3 (T0 bf16 with D*u folded on the diag,
    T1..T3 in fp8 paired into DoubleRow matmuls)
  - inter-chunk: rank-128 state (Re/Im of 64 modes), gathered by AA, advanced
    by MT, applied by VV -- all fp8 DoubleRow pairs at 2x PE throughput
  - conv weights scaled by SY=64 to stay in fp8-normal range; the PSUM result
    is descaled by 1/SY during the PSUM->SBUF bf16 copy (exact power of 2)
I/O is bf16 (halves HBM traffic); the conv term is only ~5% of output energy,
so fp8 conv + bf16 I/O keeps rel_l2 error ~3e-3 vs the 2e-2 gate.

Sharding: data-parallel over batch; each of 8 cores gets 512 sequences.
Host packs the shard time-tile-major: dev[p, j*BC + b] = u[j*128 + p, b].
"""

import os
import sys

for _p in ("/opt/trn_rl_repo",):
    if _p not in sys.path and os.path.isdir(_p):
        sys.path.append(_p)

import numpy as np
import ml_dtypes

BF16 = ml_dtypes.bfloat16
FP8 = ml_dtypes.float8_e4m3
EPS = 1e-7          # complex_softmax eps
B, L, N = 4096, 4096, 64
N_CORES = 8
BC = B // N_CORES   # 512 sequences per core
C = 512             # timesteps per chunk
NB = L // C         # 8 chunks
P = 128             # partitions
H = C // P          # 4 tiles per chunk
NT = L // P         # 32 time tiles per core
SY = 64.0           # fp8 conv-weight scale (power of 2, descaled at copy-out)
NPAIR = 8           # fp8 DoubleRow weight pairs

_PROG = None


def _constants(Lambda_re, Lambda_im, W, D, log_step):
    """Returns (T0*SY bf16 [P,P], fp8 pair block [P, NPAIR*2*P])."""
    step = float(np.exp(np.float64(log_step[0])))
    Lam = Lambda_re.astype(np.float64) + 1j * Lambda_im.astype(np.float64)
    Wc = W[0, :, 0].astype(np.float64) + 1j * W[0, :, 1].astype(np.float64)
    pows = np.exp(np.outer(np.arange(C + 1, dtype=np.float64), step * Lam))
    Gamma = pows[C]
    sl = np.arange(L, dtype=np.float64)
    Sigma = np.exp(np.outer(sl, step * Lam)).sum(axis=0)
    wt = (Wc / Lam) * np.conj(Sigma) / (Sigma * np.conj(Sigma) + EPS)
    K = (pows[:C] * wt[None, :]).sum(axis=1).real                  # (C,)

    idx = np.arange(P)
    qp = idx[None, :] - idx[:, None]
    T0 = np.where(qp >= 0, K[np.clip(qp, 0, C - 1)], 0.0) + np.eye(P) * np.float64(D[0])
    Tk = {k: K[qp + k * P] for k in range(1, H)}                   # T1..T3
    AP_ = pows[C - 1 - np.arange(C)]                               # (C, N)
    AA = np.concatenate([AP_.real, AP_.imag], axis=1)              # (C, 128)
    AAg = [AA[g * P:(g + 1) * P] for g in range(H)]
    MT = np.zeros((P, P), dtype=np.float64)
    n = np.arange(N)
    MT[n, n] = Gamma.real
    MT[64 + n, n] = -Gamma.imag
    MT[n, 64 + n] = Gamma.imag
    MT[64 + n, 64 + n] = Gamma.real
    Vq = pows[1:C + 1] * wt[None, :]                               # (C, N)
    VV = np.concatenate([Vq.real.T, -Vq.imag.T], axis=0)           # (128, C)
    VVh = [VV[:, h * P:(h + 1) * P] for h in range(H)]
    Z = np.zeros((P, P))

    # DoubleRow pairs [ktile0 | ktile1]; ktile0 multiplies rhs ktile0.
    pairs = [
        (VVh[1] * SY, Tk[1] * SY),   # 0: psY1 += VV1@s + T1@u0   rhs [s,u0]
        (VVh[3] * SY, Tk[3] * SY),   # 1: psY3 += VV3@s + T3@u0   rhs [s,u0]
        (VVh[0] * SY, Z),            # 2: psY0 += VV0@s           rhs [s,u0]
        (VVh[2] * SY, Z),            # 3: psY2 += VV2@s           rhs [s,u0]
        (Tk[2] * SY, Tk[1] * SY),    # 4: psY{g} += T2@u{g-2}+T1@u{g-1}
        (AAg[0], AAg[1]),            # 5: psS += AA0@u0 + AA1@u1  rhs [u0,u1]
        (AAg[2], AAg[3]),            # 6: psS += AA2@u2 + AA3@u3  rhs [u2,u3]
        (MT, Z),                     # 7: psS += MT@s             rhs [s,u0]
    ]
    cf = np.concatenate([np.concatenate(p, axis=1) for p in pairs], axis=1)
    cb = (T0 * SY).astype(np.float32).astype(BF16)
    return cb, cf.astype(np.float32).astype(FP8)


def _build():
    import concourse.tile as tile
    from concourse import bacc, mybir
    from contextlib import ExitStack

    bf16, f32, fp8 = mybir.dt.bfloat16, mybir.dt.float32, mybir.dt.float8e4
    DR = mybir.MatmulPerfMode.DoubleRow
    nc = bacc.Bacc("TRN2", target_bir_lowering=False, debug=False,
                   num_devices=N_CORES)
    ut = nc.dram_tensor("ut", [P, NT * BC], bf16, kind="ExternalInput").ap()
    yt = nc.dram_tensor("yt", [P, NT * BC], bf16, kind="ExternalOutput").ap()
    capb = nc.dram_tensor("CONSTB", [P, P], bf16, kind="ExternalInput").ap()
    capf = nc.dram_tensor("CONSTF", [P, NPAIR * 2 * P], fp8,
                          kind="ExternalInput").ap()

    with tile.TileContext(nc) as tc, ExitStack() as ctx:
        cpool = ctx.enter_context(tc.tile_pool(name="const", bufs=1))
        upool = ctx.enter_context(tc.tile_pool(name="u", bufs=NB))
        fpool = ctx.enter_context(tc.tile_pool(name="f", bufs=3))
        ypool = ctx.enter_context(tc.tile_pool(name="y", bufs=4))
        pypool = ctx.enter_context(tc.tile_pool(name="psy", bufs=3, space="PSUM"))
        pspool = ctx.enter_context(tc.tile_pool(name="pss", bufs=2, space="PSUM"))

        # all input DMA front-loaded: consts first (gate the first matmuls),
        # then every u chunk; upool is deep enough that none of these block
        cb = cpool.tile([P, P], bf16, tag="CONSTB")
        cf = cpool.tile([P, NPAIR * 2 * P], fp8, tag="CONSTF")
        nc.scalar.dma_start(cb[:], capb[:])
        nc.scalar.dma_start(cf[:], capf[:])
        pair = [cf[:, i * 2 * P:(i + 1) * 2 * P].rearrange("p (t m) -> p t m", t=2)
                for i in range(NPAIR)]
        uts = []
        for J in range(NB):
            uJ = upool.tile([P, H * BC], bf16, tag="u")
            nc.sync.dma_start(uJ[:], ut[:, J * H * BC:(J + 1) * H * BC])
            uts.append(uJ)

        psS_prev = None
        for J in range(NB):
            first, last = (J == 0), (J == NB - 1)
            uJ = uts[J]
            us = [uJ[:, g * BC:(g + 1) * BC] for g in range(H)]

            # fp8 staging: [s8 | u0 | u1 | u2 | u3]
            F = fpool.tile([P, 5 * BC], fp8, tag="f")
            if first:
                nc.gpsimd.memset(F[:, :BC], 0.0)       # zero initial state
            elif J % 2:
                nc.vector.tensor_copy(F[:, :BC], psS_prev[:])
            else:
                nc.scalar.copy(F[:, :BC], psS_prev[:])
            nc.gpsimd.tensor_copy(F[:, BC:], uJ[:])    # u -> fp8
            rsu = F[:, 0:2 * BC].rearrange("p (t b) -> p t b", t=2)    # [s,u0]
            r01 = F[:, BC:3 * BC].rearrange("p (t b) -> p t b", t=2)   # [u0,u1]
            r12 = F[:, 2 * BC:4 * BC].rearrange("p (t b) -> p t b", t=2)
            r23 = F[:, 3 * BC:5 * BC].rearrange("p (t b) -> p t b", t=2)

            psYa = pypool.tile([P, 2 * BC], f32, tag="psy", name=f"psYa{J}")
            psYb = pypool.tile([P, 2 * BC], f32, tag="psy", name=f"psYb{J}")
            psY = [psYa[:, :BC], psYa[:, BC:], psYb[:, :BC], psYb[:, BC:]]
            psS = (pspool.tile([P, BC], f32, tag="pss", name=f"psS{J}")
                   if not last else None)

            # bf16 diagonal (D*u folded), one stationary weight
            for g in range(H):
                nc.tensor.matmul(psY[g], cb[:], us[g], start=True, stop=False)
            # state gather + advance: psS done early so the next chunk's
            # s8 copy can start while this chunk's psY pairs still run
            if not last:
                nc.tensor.matmul(psS[:], pair[5], r01, start=True, stop=False,
                                 perf_mode=DR)
                nc.tensor.matmul(psS[:], pair[6], r23, start=False, stop=False,
                                 perf_mode=DR)
                nc.tensor.matmul(psS[:], pair[7], rsu, start=False, stop=True,
                                 perf_mode=DR)
            # off-diagonal Toeplitz, one stationary weight for two matmuls
            nc.tensor.matmul(psY[2], pair[4], r01, start=False, stop=False,
                             perf_mode=DR)
            nc.tensor.matmul(psY[3], pair[4], r12, start=False, stop=False,
                             perf_mode=DR)
            # state apply + remaining off-diagonal, closing each psY bank
            nc.tensor.matmul(psY[0], pair[2], rsu, start=False, stop=True,
                             perf_mode=DR)
            nc.tensor.matmul(psY[1], pair[0], rsu, start=False, stop=True,
                             perf_mode=DR)
            nc.tensor.matmul(psY[2], pair[3], rsu, start=False, stop=True,
                             perf_mode=DR)
            nc.tensor.matmul(psY[3], pair[1], rsu, start=False, stop=True,
                             perf_mode=DR)
            psS_prev = psS

            # PSUM -> SBUF bf16 with 1/SY descale, split vector/scalar
            yw = ypool.tile([P, H * BC], bf16, tag="y")
            nc.vector.tensor_scalar_mul(yw[:, :2 * BC], psYa[:], 1.0 / SY)
            nc.scalar.mul(yw[:, 2 * BC:], psYb[:], 1.0 / SY)
            if last:
                nc.sync.dma_start(yt[:, J * H * BC:J * H * BC + 2 * BC],
                                  yw[:, :2 * BC])
                nc.scalar.dma_start(yt[:, J * H * BC + 2 * BC:(J + 1) * H * BC],
                                    yw[:, 2 * BC:])
            else:
                nc.scalar.dma_start(yt[:, J * H * BC:(J + 1) * H * BC], yw[:])

    nc.compile()
    return nc


def _program():
    global _PROG
    if _PROG is None:
        _PROG = _build()
    return _PROG


PROFILE = False
LAST_EXEC_NS = None
LAST_RESULTS = None


def _pack_u(shard):
    """(BC, L) fp32 -> (P, NT*BC) bf16, dev[p, j*BC+b] = u[b, j*128+p]."""
    ud = np.ascontiguousarray(shard.T)
    ud = ud.reshape(NT, P, BC).transpose(1, 0, 2)
    return np.ascontiguousarray(ud.reshape(P, NT * BC)).astype(BF16)


def _unpack_y(ydev):
    yd = np.asarray(ydev).astype(np.float32).reshape(P, NT, BC)
    return yd.transpose(2, 1, 0).reshape(BC, L)


def kernel(u, Lambda_re, Lambda_im, W, D, log_step):
    global LAST_EXEC_NS
    from concourse.bass_utils import run_bass_kernel_spmd

    u = np.asarray(u, dtype=np.float32)
    cb, cf = _constants(np.asarray(Lambda_re), np.asarray(Lambda_im),
                        np.asarray(W), np.asarray(D), np.asarray(log_step))
    nc = _program()

    in_maps = []
    for c in range(N_CORES):
        in_maps.append({"ut": _pack_u(u[c * BC:(c + 1) * BC, :]),
                        "CONSTB": cb, "CONSTF": cf})

    res = run_bass_kernel_spmd(nc, in_maps, list(range(N_CORES)), trace=PROFILE)
    if PROFILE:
        LAST_EXEC_NS = res.exec_time_ns
        global LAST_RESULTS
        LAST_RESULTS = res

    y = np.empty((B, L), dtype=np.float32)
    for c in range(N_CORES):
        y[c * BC:(c + 1) * BC, :] = _unpack_y(res.results[c]["yt"])
    return y


# revision 6
# speedup vs baseline: 1.6270x; 1.6270x over previous
"""Trainium2 Bass kernel for the DSS (Diagonal State Space) layer.

y = irfft(rfft(u, 2L) * rfft(K, 2L))[:L] + D*u, with K the length-L DSS kernel
derived from (Lambda, W, log_step) via a complex softmax.

Chunked diagonal-SSM scan, C=256 timesteps per chunk (2 tiles of 128):
  - intra-chunk contribution: Toeplitz-block matmuls (T0 diag with D*u folded,
    T1 upper), all bf16
  - inter-chunk contribution: rank-128 state S (Re/Im of 64 complex modes),
    updated per chunk as S' = MT.T S + AA.T u_chunk, applied as VV.T S
All I/O and matmuls in bfloat16: full PE speed, half the HBM traffic of fp32.
The conv term is only ~5% of output energy, so bf16 keeps rel_l2 ~2e-3 vs the
2e-2 gate.  All input DMA (consts + the whole 4 MB u shard) is issued up
front: SBUF holds the full shard and the DMA runs ahead of the PE.

Sharding: data-parallel over batch; each of 8 cores gets 512 sequences.
Host packs the shard time-tile-major: dev[p, j*BC + b] = u[j*128 + p, b], so
every DMA line is long and contiguous.
"""

import os
import sys

for _p in ("/opt/trn_rl_repo",):
    if _p not in sys.path and os.path.isdir(_p):
        sys.path.append(_p)

import numpy as np
import ml_dtypes

BF16 = ml_dtypes.bfloat16
EPS = 1e-7          # complex_softmax eps
B, L, N = 4096, 4096, 64
N_CORES = 8
BC = B // N_CORES   # 512 sequences per core
C = 256             # timesteps per chunk
NB = L // C         # chunks
P = 128             # partitions
H = C // P          # 128-row tiles per chunk
NT = L // P         # 32 time tiles per core

_CNAMES = tuple(f"T{k}" for k in range(H)) + tuple(f"AA{g}" for g in range(H)) \
    + tuple(f"VV{h}" for h in range(H)) + ("MT",)

_PROG = None


def _constants(Lambda_re, Lambda_im, W, D, log_step):
    """Seven 128x128 bf16 matrices, computed in float64 on host."""
    step = float(np.exp(np.float64(log_step[0])))
    Lam = Lambda_re.astype(np.float64) + 1j * Lambda_im.astype(np.float64)
    Wc = W[0, :, 0].astype(np.float64) + 1j * W[0, :, 1].astype(np.float64)
    s = np.arange(C + 1, dtype=np.float64)
    pows = np.exp(np.outer(s, step * Lam))                                   # (C+1, N)
    Gamma = pows[C]
    sl = np.arange(L, dtype=np.float64)
    Sigma = np.exp(np.outer(sl, step * Lam)).sum(axis=0)
    wt = (Wc / Lam) * np.conj(Sigma) / (Sigma * np.conj(Sigma) + EPS)
    K = (pows[:C] * wt[None, :]).sum(axis=1).real                            # (C,)

    idx = np.arange(P)
    qp = idx[None, :] - idx[:, None]
    mats = {}
    for k in range(H):
        if k == 0:
            T = np.where(qp >= 0, K[np.clip(qp, 0, C - 1)], 0.0)
            T = T + np.eye(P) * np.float64(D[0])                             # fold D*u
        else:
            T = K[qp + k * P]
        mats[f"T{k}"] = T
    AP_ = pows[C - 1 - np.arange(C)]                                         # (C, N)
    AA = np.concatenate([AP_.real, AP_.imag], axis=1)                        # (C, 128)
    for g in range(H):
        mats[f"AA{g}"] = AA[g * P:(g + 1) * P]
    MT = np.zeros((P, P), dtype=np.float64)
    n = np.arange(N)
    MT[n, n] = Gamma.real
    MT[64 + n, n] = -Gamma.imag
    MT[n, 64 + n] = Gamma.imag
    MT[64 + n, 64 + n] = Gamma.real
    mats["MT"] = MT
    Vq = pows[1:C + 1] * wt[None, :]                                         # (C, N)
    VV = np.concatenate([Vq.real.T, -Vq.imag.T], axis=0)                     # (128, C)
    for h in range(H):
        mats[f"VV{h}"] = VV[:, h * P:(h + 1) * P]
    packed = np.concatenate([mats[name] for name in _CNAMES], axis=1)
    return packed.astype(np.float32).astype(BF16)


def _build():
    import concourse.tile as tile
    from concourse import bacc, mybir
    from contextlib import ExitStack

    bf16, f32 = mybir.dt.bfloat16, mybir.dt.float32
    nc = bacc.Bacc("TRN2", target_bir_lowering=False, debug=False,
                   num_devices=N_CORES)
    ut = nc.dram_tensor("ut", [P, NT * BC], bf16, kind="ExternalInput").ap()
    yt = nc.dram_tensor("yt", [P, NT * BC], bf16, kind="ExternalOutput").ap()
    ncst = len(_CNAMES)
    cap = nc.dram_tensor("CONST", [P, ncst * P], bf16, kind="ExternalInput").ap()

    with tile.TileContext(nc) as tc, ExitStack() as ctx:
        cpool = ctx.enter_context(tc.tile_pool(name="const", bufs=1))
        upool = ctx.enter_context(tc.tile_pool(name="u", bufs=NB))
        spool = ctx.enter_context(tc.tile_pool(name="s", bufs=2))
        ypool = ctx.enter_context(tc.tile_pool(name="y", bufs=8))
        pypool = ctx.enter_context(tc.tile_pool(name="psy", bufs=3, space="PSUM"))
        pspool = ctx.enter_context(tc.tile_pool(name="pss", bufs=2, space="PSUM"))

        # all input DMA front-loaded: consts first (they gate the first
        # matmuls), then the whole u shard; upool never recycles so nothing
        # blocks and the DMA engines run ahead of the PE
        cstt = cpool.tile([P, ncst * P], bf16, tag="CONST")
        s1 = 2 * H * P
        nc.scalar.dma_start(cstt[:, :s1], cap[:, :s1])       # T + AA blocks
        nc.scalar.dma_start(cstt[:, s1:], cap[:, s1:])       # VV + MT
        ct = {name: cstt[:, k * P:(k + 1) * P] for k, name in enumerate(_CNAMES)}
        uts = []
        for J in range(NB):
            uJ = upool.tile([P, H * BC], bf16, tag="u")
            nc.sync.dma_start(uJ[:], ut[:, J * H * BC:(J + 1) * H * BC])
            uts.append(uJ)

        s_prev = None
        for J in range(NB):
            first, last = (J == 0), (J == NB - 1)
            us = [uts[J][:, g * BC:(g + 1) * BC] for g in range(H)]

            psYp = pypool.tile([P, H * BC], f32, tag="psy", name=f"psY{J}")
            psY = [psYp[:, h * BC:(h + 1) * BC] for h in range(H)]
            psS = (pspool.tile([P, BC], f32, tag="pss", name=f"psS{J}")
                   if not last else None)
            # u-dependent matmuls, grouped by stationary weight
            for k in range(H):
                for g in range(H - k):
                    nc.tensor.matmul(psY[g + k][:], ct[f"T{k}"], us[g][:],
                                     start=(k == 0), stop=(first and g == 0))
            for g in range(H):
                if not last:
                    nc.tensor.matmul(psS[:], ct[f"AA{g}"], us[g][:],
                                     start=(g == 0), stop=(first and g == H - 1))
            # state-dependent matmuls; MT first so the next scan step's input
            # (the psS->SBUF copy) is ready as early as possible
            if not first:
                if not last:
                    nc.tensor.matmul(psS[:], ct["MT"], s_prev[:],
                                     start=False, stop=True)
                for h in range(H):
                    nc.tensor.matmul(psY[h][:], ct[f"VV{h}"], s_prev[:],
                                     start=False, stop=True)
            if not last:
                s_new = spool.tile([P, BC], bf16, tag="s")
                nc.vector.tensor_copy(s_new[:], psS[:])
                s_prev = s_new

            # PSUM -> SBUF bf16 downconvert, split across vector + scalar
            yw = ypool.tile([P, H * BC], bf16, tag="y")
            nc.vector.tensor_copy(yw[:, :BC], psYp[:, :BC])
            nc.scalar.copy(yw[:, BC:], psYp[:, BC:])
            if last:
                # split the final stores across both HWDGE queues to shorten
                # the kernel tail
                nc.sync.dma_start(yt[:, J * H * BC:J * H * BC + BC], yw[:, :BC])
                nc.scalar.dma_start(yt[:, J * H * BC + BC:(J + 1) * H * BC],
                                    yw[:, BC:])
            else:
                nc.scalar.dma_start(yt[:, J * H * BC:(J + 1) * H * BC], yw[:])

    nc.compile()
    return nc


def _program():
    global _PROG
    if _PROG is None:
        _PROG = _build()
    return _PROG


PROFILE = False
LAST_EXEC_NS = None
LAST_RESULTS = None


def _pack_u(shard):
    """(BC, L) fp32 -> (P, NT*BC) bf16, dev[p, j*BC+b] = u[b, j*128+p]."""
    ud = np.ascontiguousarray(shard.T)
    ud = ud.reshape(NT, P, BC).transpose(1, 0, 2)
    return np.ascontiguousarray(ud.reshape(P, NT * BC)).astype(BF16)


def _unpack_y(ydev):
    yd = np.asarray(ydev).astype(np.float32).reshape(P, NT, BC)
    return yd.transpose(2, 1, 0).reshape(BC, L)


def kernel(u, Lambda_re, Lambda_im, W, D, log_step):
    global LAST_EXEC_NS
    from concourse.bass_utils import run_bass_kernel_spmd

    u = np.asarray(u, dtype=np.float32)
    consts = _constants(np.asarray(Lambda_re), np.asarray(Lambda_im),
                        np.asarray(W), np.asarray(D), np.asarray(log_step))
    nc = _program()

    in_maps = []
    for c in range(N_CORES):
        in_maps.append({"ut": _pack_u(u[c * BC:(c + 1) * BC, :]),
                        "CONST": consts})

    res = run_bass_kernel_spmd(nc, in_maps, list(range(N_CORES)), trace=PROFILE)
    if PROFILE:
        LAST_EXEC_NS = res.exec_time_ns
        global LAST_RESULTS
        LAST_RESULTS = res

    y = np.empty((B, L), dtype=np.float32)
    for c in range(N_CORES):
        y[c * BC:(c + 1) * BC, :] = _unpack_y(res.results[c]["yt"])
    return y
